# revision 19
# baseline (speedup 1.0000x reference)
"""Trainium2 Bass kernel for CustomizeLSTMCell (fused 4-matmul LSTM-like cell).

Math (per token row x of N=100000, H=150):
    pre    = s_in @ W_in + s_out @ W_out + h_in @ U_in + h_out @ U_out
    gate   = sigmoid(pre)
    cell   = gate * last_c + gate * gate = gate * (last_c + gate)
    hidden = gate * tanh(cell)
returns (hidden, cell)

Strategy: data-parallel over tokens across 8 cores (12500 rows/core, padded
to 12544), feature-major on chip with the 150 output features split 75/75
(A/B halves).

Mixed-precision input encoding (the kernel is DMA-bound, 360 GB/s/core):
the 600 contraction features are split 360 fp16 + 240 fp8 (e3m4). The fp8
subset is chosen at runtime as the 240 weight rows with the SMALLEST L2
norm, which minimizes the quantization error injected into the
pre-activations (measured absmax-scaled output error 1.63e-2 vs the 2e-2
gate; naive last-240 split gives 1.90e-2). ALL weights stay fp16 (w rows
are ~0.05*randn — entirely subnormal in e3m4 — and mixed-dtype matmul,
fp16 stationary x fp8 moving, runs at 1 cycle/row). Traffic drops from
2100B/token (all-fp16) to 1860B/token: DMA floor ~65.5us vs 73.4us.

Queue layout (the critical part — three DMA-capable queues):
  SP:   ALL loads (x16, x8, c) + the end-of-program deferred stores. Loads
        depend only on deep pool rotation, so the SP stream never stalls:
        a c-load goes to its own cpool tile (NOT into the o_tile, whose
        buffer rotation depends on stores).
  Pool: per-macro output stores (SWDGE). A store waits on its macro's
        compute chain; head-of-line blocking here is harmless because
        store k+1's chain finishes after store k's.
  ACT:  activations only (sigmoid/tanh) — a DMA dispatch on ACT would hold
        the ACT sequencer during its dependency wait and stall the chain.

Software-pipelined epilogue: tanh + hidden-muls + store of macro k-1 issue
during macro k, so every ACT instruction's inputs are long-ready and ACT
streams bubble-free (~4.2us/macro); it is the pipeline pacer, slightly
behind PE (4.27us/macro real matmul work).

Endgame: the first N_DEFER macros' stores go to a SEPARATE DRAM tensor
(out2) and are issued at the end of the program — a ~8.5us bank of
ready-to-fire traffic that keeps the DMA engines busy while the final
chains drain. (A single DRAM output tensor would serialize the deferred
stores behind the last in-loop store.) The tail macros are 512/512/256 so
the last chain-gated stores are small and arrive early. Ideal end-to-end:
startup ~1.9us + DMA busy ~65.5us + final sem 0.9us.

PE p-state: an idle PE drops to 0.65-1.2 GHz and needs 3us of continuous
execution to return to 2.4 GHz; warmup fillers bridge PE from t=0 to the
first real matmul, after which the deep load prefetch keeps PE busy with
real work.
"""

import numpy as np

N_TOKENS = 100000
UNITS = 150
N_CORES = 8
ROWS_PER_CORE = N_TOKENS // N_CORES  # 12500
ROWS_PAD = 12544                     # 11*1024 + 512 + 512 + 256
# small tail macros so the last chain-gated stores are small and arrive early
MACROS = [1024] * 11 + [512, 512, 256]
TILE = 512                           # matmul free-dim (= one PSUM bank of fp32)
KDIM = 4 * UNITS                     # 600
KCHUNK = 120
K16 = 360                            # fp16 features (3 chunks of 120)
K8 = 240                             # fp8 features (2 chunks of 120)
N_K16 = K16 // KCHUNK                # 3
N_K8 = K8 // KCHUNK                  # 2
N_KCHUNKS = N_K16 + N_K8             # 5
MHALF = 75                           # feature half (A: 0:75, B: 75:150)
N_DEFER = 5
DEFER_ROWS = sum(MACROS[:N_DEFER])   # tokens covered by the deferred macros

_CACHE = {}
REPS = 1  # timing aid: repeat the whole macro loop (outputs are idempotent)


def _build_bass():
    import concourse.bacc as bacc
    import concourse.mybir as mybir
    import concourse.tile as tile

    fp32 = mybir.dt.float32
    fp16 = mybir.dt.float16
    fp8 = mybir.dt.float8e3
    nc = bacc.Bacc("TRN2", target_bir_lowering=False, debug=False,
                   num_devices=N_CORES)

    x16 = nc.dram_tensor("x16", [K16, ROWS_PAD], fp16,
                         kind="ExternalInput").ap()
    x8 = nc.dram_tensor("x8", [K8, ROWS_PAD], fp8,
                        kind="ExternalInput").ap()
    c = nc.dram_tensor("c", [MHALF, 2, ROWS_PAD], fp16,
                       kind="ExternalInput").ap()
    # single fp16 weight tensor: chunks 0:3 pair with x16, chunks 3:5 with x8
    w = nc.dram_tensor("w", [KCHUNK, N_KCHUNKS * UNITS], fp16,
                       kind="ExternalInput").ap()
    out = nc.dram_tensor("out", [MHALF, 4, ROWS_PAD], fp16,
                         kind="ExternalOutput").ap()
    out2 = nc.dram_tensor("out2", [MHALF, 4, DEFER_ROWS], fp16,
                          kind="ExternalOutput").ap()

    AF = mybir.ActivationFunctionType

    x16_r = x16.rearrange("(k p) t -> p k t", p=KCHUNK)    # [120, 3, 12544]
    x8_r = x8.rearrange("(k p) t -> p k t", p=KCHUNK)      # [120, 2, 12544]
    w_r = w.rearrange("p (k d) -> p k d", k=N_KCHUNKS)     # [120, 5, 150]

    with tile.TileContext(nc) as tc:
        with (
            tc.tile_pool(name="wpool", bufs=1) as wpool,
            # DEEP prefetch: every macro's loads stream back-to-back at full
            # DMA rate; PE (slower per macro than the load stream) never
            # starves, so its p-state streak is unbroken without fillers.
            tc.tile_pool(name="x16pool", bufs=8) as x16pool,
            tc.tile_pool(name="x8pool", bufs=8) as x8pool,
            tc.tile_pool(name="cpool", bufs=6) as cpool,
            tc.tile_pool(name="opool", bufs=4) as opool,
            tc.tile_pool(name="odef", bufs=1) as odef_pool,
            tc.tile_pool(name="gpool", bufs=4) as gpool,
            # per-(half) PSUM tiles (2 banks each), 3 rotating bufs: deep
            # enough that matmuls never wait on sigmoid drain.
            tc.tile_pool(name="psum", bufs=3, space="PSUM") as psum_pool,
            tc.tile_pool(name="fill", bufs=1) as fill_pool,
            tc.tile_pool(name="fpsum", bufs=1, space="PSUM") as fpsum_pool,
        ):
            w_tile = wpool.tile([KCHUNK, N_KCHUNKS, UNITS], fp16)
            nc.sync.dma_start(w_tile[:, :, :], w_r[:, :, :])

            # PE p-state warming (see module docstring). The memset runs on
            # the otherwise-idle Pool engine so the first filler can start
            # ~0.6us in (a DVE memset would push it past 1.4us).
            fx = fill_pool.tile([KCHUNK, TILE], fp16)
            nc.gpsimd.memset(fx[:, :], 0.0)

            def pe_filler(n):
                for _ in range(n):
                    fp = fpsum_pool.tile([MHALF, TILE], fp32)
                    nc.tensor.matmul(fp[:, :], lhsT=fx[:, 0:MHALF],
                                     rhs=fx[:, :], start=True, stop=True)

            pe_filler(7)

            deferred = []
            pending = []

            def _finalize(item):
                o_tile, gate, flo, fhi, fmsz, fdefer = item
                if not fdefer and fmsz < 1024:
                    # tail macros: per-half chains and stores, so each half's
                    # store fires as soon as its own hidden-mul lands --
                    # finer, earlier arrivals exactly where the DMA drains
                    for f in range(2):
                        nc.scalar.activation(o_tile[:, 2 * f, 0:fmsz],
                                             o_tile[:, 2 * f + 1, 0:fmsz],
                                             AF.Tanh)
                        hid = o_tile[:, 2 * f, 0:fmsz]
                        nc.vector.tensor_mul(hid, gate[:, f, 0:fmsz], hid)
                        nc.gpsimd.dma_start(
                            out[:, 2 * f:2 * f + 2, flo:fhi],
                            o_tile[:, 2 * f:2 * f + 2, 0:fmsz])
                    return
                for f in range(2):
                    nc.scalar.activation(o_tile[:, 2 * f, 0:fmsz],
                                         o_tile[:, 2 * f + 1, 0:fmsz],
                                         AF.Tanh)
                for f in range(2):
                    hid = o_tile[:, 2 * f, 0:fmsz]
                    nc.vector.tensor_mul(hid, gate[:, f, 0:fmsz], hid)
                if fdefer:
                    deferred.append((o_tile, flo, fhi, fmsz))
                else:
                    # Pool/SWDGE queue: a store waiting on its chain must not
                    # head-of-line block ACT activations or SP loads.
                    nc.gpsimd.dma_start(out[:, :, flo:fhi],
                                        o_tile[:, :, 0:fmsz])

            macros = [m for _ in range(REPS) for m in MACROS]
            lo = 0
            for rep_i, msz in enumerate(macros):
                if rep_i > 0 and lo + msz > ROWS_PAD:
                    lo = 0
                hi = lo + msz
                ntile = (msz + TILE - 1) // TILE
                defer = rep_i < N_DEFER

                x16_tile = x16pool.tile([KCHUNK, N_K16, 1024], fp16)
                x8_tile = x8pool.tile([KCHUNK, N_K8, 1024], fp8)
                if rep_i == 0:
                    # split loads: the first matmul starts as soon as
                    # chunk 0 lands (~3.2us), pulling the whole chain earlier
                    nc.sync.dma_start(x16_tile[:, 0, 0:msz],
                                      x16_r[:, 0, lo:hi])
                    nc.sync.dma_start(x16_tile[:, 1:, 0:msz],
                                      x16_r[:, 1:, lo:hi])
                    nc.sync.dma_start(x8_tile[:, :, 0:msz],
                                      x8_r[:, :, lo:hi])
                else:
                    nc.sync.dma_start(x16_tile[:, :, 0:msz],
                                      x16_r[:, :, lo:hi])
                    nc.sync.dma_start(x8_tile[:, :, 0:msz],
                                      x8_r[:, :, lo:hi])
                c_tile = cpool.tile([MHALF, 2, 1024], fp16)
                nc.sync.dma_start(c_tile[:, :, 0:msz], c[:, :, lo:hi])

                # o_tile cols: [0]=h_A, [1]=cell_A, [2]=h_B, [3]=cell_B
                if defer:
                    o_tile = odef_pool.tile([MHALF, 4, 1024], fp16,
                                            tag=f"od{rep_i}")
                else:
                    o_tile = opool.tile([MHALF, 4, 1024], fp16)

                gate = gpool.tile([MHALF, 2, 1024], fp16)

                def half(f):
                    fs = slice(f * MHALF, (f + 1) * MHALF)
                    pre = psum_pool.tile([MHALF, 1024], fp32)
                    for t in range(ntile):
                        t0, t1 = t * TILE, min((t + 1) * TILE, msz)
                        for k in range(N_K16):
                            nc.tensor.matmul(
                                pre[:, t0:t1],
                                lhsT=w_tile[:, k, fs],
                                rhs=x16_tile[:, k, t0:t1],
                                start=(k == 0),
                                stop=False,
                            )
                        for k in range(N_K8):
                            nc.tensor.matmul(
                                pre[:, t0:t1],
                                lhsT=w_tile[:, N_K16 + k, fs],
                                rhs=x8_tile[:, k, t0:t1],
                                start=False,
                                stop=(k == N_K8 - 1),
                            )
                    nc.scalar.activation(gate[:, f, 0:msz],
                                         pre[:, 0:msz], AF.Sigmoid)
                    cell = o_tile[:, 2 * f + 1, 0:msz]
                    nc.vector.tensor_add(cell, c_tile[:, f, 0:msz],
                                         gate[:, f, 0:msz])
                    nc.vector.tensor_mul(cell, gate[:, f, 0:msz], cell)

                # Software-pipelined epilogue, interleaved between the two
                # half-passes: the ACT queue sees [sigA(k), tanh(k-1) x2,
                # sigB(k)], so ACT chews long-ready tanh work while PE
                # finishes the B half (no phase wait on sigB), and macro
                # k-1's store chain completes ~1us earlier.
                half(0)
                if pending:
                    _finalize(pending.pop(0))
                half(1)
                pending.append((o_tile, gate, lo, hi, msz, defer))
                lo = hi

            _finalize(pending.pop(0))

            # Deferred-store bank: ready the moment they dispatch; they keep
            # the DMA engines busy while the final chains drain.
            for o_tile, dlo, dhi, dmsz in deferred:
                nc.sync.dma_start(out2[:, :, dlo:dhi], o_tile[:, :, 0:dmsz])

    nc.compile()
    return nc


def _get_nc():
    if "nc" not in _CACHE:
        _CACHE["nc"] = _build_bass()
    return _CACHE["nc"]


def kernel(s_in, s_out, h_in, h_out, last_c,
           w_in_input, w_out_input, u_in_input, u_out_input):
    import ml_dtypes
    from concourse.bass_utils import run_bass_kernel_spmd

    nc = _get_nc()

    f16 = np.float16
    f8 = ml_dtypes.float8_e3m4

    wcat = np.concatenate(
        [w_in_input, w_out_input, u_in_input, u_out_input],
        axis=0).astype(np.float32)                      # [600, 150]
    # fp8 feature subset: the 240 weight rows with smallest L2 norm inject
    # the least quantization error into pre (see module docstring).
    row_norms = (wcat.astype(np.float64) ** 2).sum(axis=1)
    order = np.argsort(row_norms, kind="stable")
    perm8 = np.sort(order[:K8])                         # 240 features -> fp8
    perm16 = np.sort(order[K8:])                        # 360 features -> fp16

    # w[p, k*150+d] = wcat[perm[k*120+p], d] with perm = perm16 ++ perm8.
    perm = np.concatenate([perm16, perm8])
    wp = np.ascontiguousarray(
        wcat[perm].reshape(N_KCHUNKS, KCHUNK, UNITS).transpose(1, 0, 2)
        .reshape(KCHUNK, N_KCHUNKS * UNITS)).astype(f16)

    xcat = np.concatenate(
        [np.asarray(a) for a in (s_in, s_out, h_in, h_out)],
        axis=1)                                          # [N, 600] fp32

    in_maps = []
    for core in range(N_CORES):
        rows = slice(core * ROWS_PER_CORE, (core + 1) * ROWS_PER_CORE)
        x16T = np.zeros((K16, ROWS_PAD), dtype=f16)
        x16T[:, :ROWS_PER_CORE] = xcat[rows][:, perm16].T.astype(f16)
        x8T = np.zeros((K8, ROWS_PAD), dtype=f8)
        x8T[:, :ROWS_PER_CORE] = xcat[rows][:, perm8].T.astype(f8)
        cp = np.zeros((MHALF, 2, ROWS_PAD), dtype=f16)
        cT = np.asarray(last_c[rows]).T.astype(f16)     # [150, 12500]
        cp[:, 0, :ROWS_PER_CORE] = cT[:MHALF]
        cp[:, 1, :ROWS_PER_CORE] = cT[MHALF:]
        in_maps.append({"x16": x16T, "x8": x8T, "c": cp, "w": wp})

    res = run_bass_kernel_spmd(nc, in_maps, core_ids=list(range(N_CORES)))

    hidden = np.empty((N_TOKENS, UNITS), dtype=np.float32)
    cell = np.empty((N_TOKENS, UNITS), dtype=np.float32)
    for core in range(N_CORES):
        rows = slice(core * ROWS_PER_CORE, (core + 1) * ROWS_PER_CORE)
        o = np.concatenate(
            [res.results[core]["out2"],
             res.results[core]["out"][:, :, DEFER_ROWS:]],
            axis=2)[:, :, :ROWS_PER_CORE]               # [75, 4, 12500]
        hidden[rows, :MHALF] = o[:, 0, :].T
        hidden[rows, MHALF:] = o[:, 2, :].T
        cell[rows, :MHALF] = o[:, 1, :].T
        cell[rows, MHALF:] = o[:, 3, :].T
    return hidden, cell


# revision 21
# speedup vs baseline: 1.0268x; 1.0268x over previous
"""Trainium2 Bass kernel for CustomizeLSTMCell (fused 4-matmul LSTM-like cell).

Math (per token row x of N=100000, H=150):
    pre    = s_in @ W_in + s_out @ W_out + h_in @ U_in + h_out @ U_out
    gate   = sigmoid(pre)
    cell   = gate * last_c + gate * gate = gate * (last_c + gate)
    hidden = gate * tanh(cell)
returns (hidden, cell)

Strategy: data-parallel over tokens across 8 cores (12500 rows/core, padded
to 12544), feature-major on chip with the 150 output features split 75/75
(A/B halves).

Mixed-precision input encoding (the kernel is DMA-bound, 360 GB/s/core):
the 600 contraction features are split 360 fp16 + 240 fp8 (e3m4). The fp8
subset is chosen at runtime as the 240 weight rows with the SMALLEST L2
norm, which minimizes the quantization error injected into the
pre-activations (measured absmax-scaled output error 1.63e-2 vs the 2e-2
gate; naive last-240 split gives 1.90e-2). ALL weights stay fp16 (w rows
are ~0.05*randn — entirely subnormal in e3m4 — and mixed-dtype matmul,
fp16 stationary x fp8 moving, runs at 1 cycle/row). Traffic drops from
2100B/token (all-fp16) to 1860B/token: DMA floor ~65.5us vs 73.4us.

Queue layout (the critical part — three DMA-capable queues):
  SP:   ALL loads (x16, x8, c) + the end-of-program deferred stores. Loads
        depend only on deep pool rotation, so the SP stream never stalls:
        a c-load goes to its own cpool tile (NOT into the o_tile, whose
        buffer rotation depends on stores).
  Pool: per-macro output stores (SWDGE). A store waits on its macro's
        compute chain; head-of-line blocking here is harmless because
        store k+1's chain finishes after store k's.
  ACT:  activations only (sigmoid/tanh) — a DMA dispatch on ACT would hold
        the ACT sequencer during its dependency wait and stall the chain.

Software-pipelined epilogue: tanh + hidden-muls + store of macro k-1 issue
during macro k, so every ACT instruction's inputs are long-ready and ACT
streams bubble-free (~4.2us/macro); it is the pipeline pacer, slightly
behind PE (4.27us/macro real matmul work).

Endgame: the first N_DEFER macros' stores go to a SEPARATE DRAM tensor
(out2) and are issued at the end of the program — a ~8.5us bank of
ready-to-fire traffic that keeps the DMA engines busy while the final
chains drain. (A single DRAM output tensor would serialize the deferred
stores behind the last in-loop store.) The tail macros are 512/512/256 so
the last chain-gated stores are small and arrive early. Ideal end-to-end:
startup ~1.9us + DMA busy ~65.5us + final sem 0.9us.

PE p-state: an idle PE drops to 0.65-1.2 GHz and needs 3us of continuous
execution to return to 2.4 GHz; warmup fillers bridge PE from t=0 to the
first real matmul, after which the deep load prefetch keeps PE busy with
real work.
"""

import numpy as np

N_TOKENS = 100000
UNITS = 150
N_CORES = 8
ROWS_PER_CORE = N_TOKENS // N_CORES  # 12500
ROWS_PAD = 12544                     # 11*1024 + 512 + 512 + 256
# small tail macros so the last chain-gated stores are small and arrive early
MACROS = [1024] * 11 + [512, 512, 256]
TILE = 512                           # matmul free-dim (= one PSUM bank of fp32)
KDIM = 4 * UNITS                     # 600
KCHUNK = 120
K16 = 360                            # fp16 features (3 chunks of 120)
K8 = 240                             # fp8 features (2 chunks of 120)
N_K16 = K16 // KCHUNK                # 3
N_K8 = K8 // KCHUNK                  # 2
N_KCHUNKS = N_K16 + N_K8             # 5
MHALF = 75                           # feature half (A: 0:75, B: 75:150)
N_DEFER = 5
DEFER_ROWS = sum(MACROS[:N_DEFER])   # tokens covered by the deferred macros

_CACHE = {}
REPS = 1  # timing aid: repeat the whole macro loop (outputs are idempotent)


def _build_bass():
    import concourse.bacc as bacc
    import concourse.mybir as mybir
    import concourse.tile as tile

    fp32 = mybir.dt.float32
    fp16 = mybir.dt.float16
    fp8 = mybir.dt.float8e3
    nc = bacc.Bacc("TRN2", target_bir_lowering=False, debug=False,
                   num_devices=N_CORES)

    x16 = nc.dram_tensor("x16", [K16, ROWS_PAD], fp16,
                         kind="ExternalInput").ap()
    x8 = nc.dram_tensor("x8", [K8, ROWS_PAD], fp8,
                        kind="ExternalInput").ap()
    c = nc.dram_tensor("c", [MHALF, 2, ROWS_PAD], fp16,
                       kind="ExternalInput").ap()
    # single fp16 weight tensor: chunks 0:3 pair with x16, chunks 3:5 with x8
    w = nc.dram_tensor("w", [KCHUNK, N_KCHUNKS * UNITS], fp16,
                       kind="ExternalInput").ap()
    out = nc.dram_tensor("out", [MHALF, 4, ROWS_PAD], fp16,
                         kind="ExternalOutput").ap()
    out2 = nc.dram_tensor("out2", [MHALF, 4, DEFER_ROWS], fp16,
                          kind="ExternalOutput").ap()

    AF = mybir.ActivationFunctionType

    x16_r = x16.rearrange("(k p) t -> p k t", p=KCHUNK)    # [120, 3, 12544]
    x8_r = x8.rearrange("(k p) t -> p k t", p=KCHUNK)      # [120, 2, 12544]
    w_r = w.rearrange("p (k d) -> p k d", k=N_KCHUNKS)     # [120, 5, 150]

    with tile.TileContext(nc) as tc:
        with (
            tc.tile_pool(name="wpool", bufs=1) as wpool,
            # DEEP prefetch: every macro's loads stream back-to-back at full
            # DMA rate; PE (slower per macro than the load stream) never
            # starves, so its p-state streak is unbroken without fillers.
            tc.tile_pool(name="x16pool", bufs=8) as x16pool,
            tc.tile_pool(name="x8pool", bufs=8) as x8pool,
            tc.tile_pool(name="cpool", bufs=6) as cpool,
            tc.tile_pool(name="opool", bufs=4) as opool,
            tc.tile_pool(name="odef", bufs=1) as odef_pool,
            tc.tile_pool(name="gpool", bufs=4) as gpool,
            # per-(half) PSUM tiles (2 banks each), 3 rotating bufs: deep
            # enough that matmuls never wait on sigmoid drain.
            tc.tile_pool(name="psum", bufs=3, space="PSUM") as psum_pool,
            tc.tile_pool(name="fill", bufs=1) as fill_pool,
            tc.tile_pool(name="fpsum", bufs=1, space="PSUM") as fpsum_pool,
        ):
            w_tile = wpool.tile([KCHUNK, N_KCHUNKS, UNITS], fp16)
            nc.sync.dma_start(w_tile[:, :, :], w_r[:, :, :])

            # PE p-state warming (see module docstring). The memset runs on
            # the otherwise-idle Pool engine so the first filler can start
            # ~0.6us in (a DVE memset would push it past 1.4us).
            fx = fill_pool.tile([KCHUNK, TILE], fp16)
            nc.gpsimd.memset(fx[:, :], 0.0)

            def pe_filler(n):
                for _ in range(n):
                    fp = fpsum_pool.tile([MHALF, TILE], fp32)
                    nc.tensor.matmul(fp[:, :], lhsT=fx[:, 0:MHALF],
                                     rhs=fx[:, :], start=True, stop=True)

            pe_filler(7)

            deferred = []
            pending = []

            def _finalize(item, last=False):
                o_tile, gate, flo, fhi, fmsz, fdefer = item
                for f in range(2):
                    nc.scalar.activation(o_tile[:, 2 * f, 0:fmsz],
                                         o_tile[:, 2 * f + 1, 0:fmsz],
                                         AF.Tanh)
                for f in range(2):
                    hid = o_tile[:, 2 * f, 0:fmsz]
                    nc.vector.tensor_mul(hid, gate[:, f, 0:fmsz], hid)
                if fdefer:
                    deferred.append((o_tile, flo, fhi, fmsz))
                elif last:
                    # the final stores ride the ACT HWDGE queue: ACT has just
                    # drained (nothing follows, so no head-of-line risk) and
                    # its HWDGE path beats Pool's SWDGE desc-gen + queueing,
                    # shaving the post-chain latency of the kernel's very
                    # last transfers
                    nc.scalar.dma_start(out[:, :, flo:fhi],
                                        o_tile[:, :, 0:fmsz])
                else:
                    # Pool/SWDGE queue: a store waiting on its chain must not
                    # head-of-line block ACT activations or SP loads.
                    nc.gpsimd.dma_start(out[:, :, flo:fhi],
                                        o_tile[:, :, 0:fmsz])

            macros = [m for _ in range(REPS) for m in MACROS]
            lo = 0
            for rep_i, msz in enumerate(macros):
                if rep_i > 0 and lo + msz > ROWS_PAD:
                    lo = 0
                hi = lo + msz
                ntile = (msz + TILE - 1) // TILE
                defer = rep_i < N_DEFER

                x16_tile = x16pool.tile([KCHUNK, N_K16, 1024], fp16)
                x8_tile = x8pool.tile([KCHUNK, N_K8, 1024], fp8)
                if rep_i == 0:
                    # split loads: the first matmul starts as soon as
                    # chunk 0 lands (~3.2us), pulling the whole chain earlier
                    nc.sync.dma_start(x16_tile[:, 0, 0:msz],
                                      x16_r[:, 0, lo:hi])
                    nc.sync.dma_start(x16_tile[:, 1:, 0:msz],
                                      x16_r[:, 1:, lo:hi])
                    nc.sync.dma_start(x8_tile[:, :, 0:msz],
                                      x8_r[:, :, lo:hi])
                else:
                    nc.sync.dma_start(x16_tile[:, :, 0:msz],
                                      x16_r[:, :, lo:hi])
                    nc.sync.dma_start(x8_tile[:, :, 0:msz],
                                      x8_r[:, :, lo:hi])
                c_tile = cpool.tile([MHALF, 2, 1024], fp16)
                nc.sync.dma_start(c_tile[:, :, 0:msz], c[:, :, lo:hi])

                # o_tile cols: [0]=h_A, [1]=cell_A, [2]=h_B, [3]=cell_B
                if defer:
                    o_tile = odef_pool.tile([MHALF, 4, 1024], fp16,
                                            tag=f"od{rep_i}")
                else:
                    o_tile = opool.tile([MHALF, 4, 1024], fp16)

                gate = gpool.tile([MHALF, 2, 1024], fp16)

                def half(f):
                    fs = slice(f * MHALF, (f + 1) * MHALF)
                    pre = psum_pool.tile([MHALF, 1024], fp32)
                    for t in range(ntile):
                        t0, t1 = t * TILE, min((t + 1) * TILE, msz)
                        for k in range(N_K16):
                            nc.tensor.matmul(
                                pre[:, t0:t1],
                                lhsT=w_tile[:, k, fs],
                                rhs=x16_tile[:, k, t0:t1],
                                start=(k == 0),
                                stop=False,
                            )
                        for k in range(N_K8):
                            nc.tensor.matmul(
                                pre[:, t0:t1],
                                lhsT=w_tile[:, N_K16 + k, fs],
                                rhs=x8_tile[:, k, t0:t1],
                                start=False,
                                stop=(k == N_K8 - 1),
                            )
                    nc.scalar.activation(gate[:, f, 0:msz],
                                         pre[:, 0:msz], AF.Sigmoid)
                    cell = o_tile[:, 2 * f + 1, 0:msz]
                    nc.vector.tensor_add(cell, c_tile[:, f, 0:msz],
                                         gate[:, f, 0:msz])
                    nc.vector.tensor_mul(cell, gate[:, f, 0:msz], cell)

                # Software-pipelined epilogue, interleaved between the two
                # half-passes: the ACT queue sees [sigA(k), tanh(k-1) x2,
                # sigB(k)], so ACT chews long-ready tanh work while PE
                # finishes the B half (no phase wait on sigB), and macro
                # k-1's store chain completes ~1us earlier.
                half(0)
                if pending:
                    _finalize(pending.pop(0), last=(rep_i == len(macros) - 1))
                half(1)
                pending.append((o_tile, gate, lo, hi, msz, defer))
                lo = hi

            _finalize(pending.pop(0), last=True)

            # Deferred-store bank: ready the moment they dispatch; they keep
            # the DMA engines busy while the final chains drain.
            for o_tile, dlo, dhi, dmsz in deferred:
                nc.sync.dma_start(out2[:, :, dlo:dhi], o_tile[:, :, 0:dmsz])

    nc.compile()
    return nc


def _get_nc():
    if "nc" not in _CACHE:
        _CACHE["nc"] = _build_bass()
    return _CACHE["nc"]


def kernel(s_in, s_out, h_in, h_out, last_c,
           w_in_input, w_out_input, u_in_input, u_out_input):
    import ml_dtypes
    from concourse.bass_utils import run_bass_kernel_spmd

    nc = _get_nc()

    f16 = np.float16
    f8 = ml_dtypes.float8_e3m4

    wcat = np.concatenate(
        [w_in_input, w_out_input, u_in_input, u_out_input],
        axis=0).astype(np.float32)                      # [600, 150]
    # fp8 feature subset: the 240 weight rows with smallest L2 norm inject
    # the least quantization error into pre (see module docstring).
    row_norms = (wcat.astype(np.float64) ** 2).sum(axis=1)
    order = np.argsort(row_norms, kind="stable")
    perm8 = np.sort(order[:K8])                         # 240 features -> fp8
    perm16 = np.sort(order[K8:])                        # 360 features -> fp16

    # w[p, k*150+d] = wcat[perm[k*120+p], d] with perm = perm16 ++ perm8.
    perm = np.concatenate([perm16, perm8])
    wp = np.ascontiguousarray(
        wcat[perm].reshape(N_KCHUNKS, KCHUNK, UNITS).transpose(1, 0, 2)
        .reshape(KCHUNK, N_KCHUNKS * UNITS)).astype(f16)

    xcat = np.concatenate(
        [np.asarray(a) for a in (s_in, s_out, h_in, h_out)],
        axis=1)                                          # [N, 600] fp32

    in_maps = []
    for core in range(N_CORES):
        rows = slice(core * ROWS_PER_CORE, (core + 1) * ROWS_PER_CORE)
        x16T = np.zeros((K16, ROWS_PAD), dtype=f16)
        x16T[:, :ROWS_PER_CORE] = xcat[rows][:, perm16].T.astype(f16)
        x8T = np.zeros((K8, ROWS_PAD), dtype=f8)
        x8T[:, :ROWS_PER_CORE] = xcat[rows][:, perm8].T.astype(f8)
        cp = np.zeros((MHALF, 2, ROWS_PAD), dtype=f16)
        cT = np.asarray(last_c[rows]).T.astype(f16)     # [150, 12500]
        cp[:, 0, :ROWS_PER_CORE] = cT[:MHALF]
        cp[:, 1, :ROWS_PER_CORE] = cT[MHALF:]
        in_maps.append({"x16": x16T, "x8": x8T, "c": cp, "w": wp})

    res = run_bass_kernel_spmd(nc, in_maps, core_ids=list(range(N_CORES)))

    hidden = np.empty((N_TOKENS, UNITS), dtype=np.float32)
    cell = np.empty((N_TOKENS, UNITS), dtype=np.float32)
    for core in range(N_CORES):
        rows = slice(core * ROWS_PER_CORE, (core + 1) * ROWS_PER_CORE)
        o = np.concatenate(
            [res.results[core]["out2"],
             res.results[core]["out"][:, :, DEFER_ROWS:]],
            axis=2)[:, :, :ROWS_PER_CORE]               # [75, 4, 12500]
        hidden[rows, :MHALF] = o[:, 0, :].T
        hidden[rows, MHALF:] = o[:, 2, :].T
        cell[rows, :MHALF] = o[:, 1, :].T
        cell[rows, MHALF:] = o[:, 3, :].T
    return hidden, cell
